# revision 3
# baseline (speedup 1.0000x reference)
"""Trainium2 Bass kernel for nn_AttentionBlock — fp8 DoubleRow version (v3).

Sharding: data-parallel over batch B=32 across 8 NeuronCores (4 per core).

vs the bf16 baseline (245us): all four per-batch matmul stages (G = B^T h,
V2^T = h^T W2^T, S^T = h^T G, out = V2 attn^T) run in fp8e4m3 with the
DoubleRow perf mode — each instruction contracts 256 (two 128-chunks packed
into the stationary) at 0.5 cycles/row, ~4x the bf16 MAC rate (measured
~240ns vs ~920ns per K=512 x [128,512] group). The 2e-2 rel gate has ~50x
headroom over bf16 (3.6e-4); measured fp8 end-to-end error is ~5e-3.

Measured ladder (slope method, same contract as the baseline's 245us):
bf16 baseline 237 -> fp8 naive 183 -> host-folded B/W2 + mega-tiles +
reorder 152 -> For_i 4-body unrolling (the tile For_i has an all-engine
barrier per iteration, ~50us; unrolling amortizes it) + pipeline-depth
fixes 149-152us -> small ACT->DVE offloads (rden-broadcast evac, GroupNorm
bc scale) ~150us in a slow-device-state process (ACT is the modeled wall;
DVE has headroom). Rel err 5.1e-3 (gate 2e-2). TimelineSim's steady-state model
says ACT is the busiest engine (~80us/110us iteration) and the
exp->denominator->rden->PV chain sets the rest of the critical path.

Structure (driven by TimelineSim: the kernel is stall- and
evacuation-bound, not matmul-bound, after fp8; GPSIMD cannot touch PSUM
and DMA cannot read PSUM, so all PSUM evacuation goes through ACT/DVE):
 - B = wq^T wk (xBSC) and W2^T = (wproj wv)^T (xW2SC) are computed on the
   HOST and shipped as fp8 — removes all setup matmuls, their PSUM
   evacuations, and 1.5MB of weight DMA from the kernel head.
 - x arrives as bf16 (halves input DMA; residual in bf16 is within budget).
 - every PSUM tile is a [P, 1024] 2-bank mega-tile (one evacuation
   instruction each): pool layout = 3x mega (6 banks) + denominator (2).
 - per-batch PE order: G -> S/exp (+ denominator ones-fp8-DoubleRow matmuls
   interleaved) -> V2 -> rden broadcast -> PV. The denominator chain
   (recip on DVE directly from PSUM -> bf16 ones-matmul broadcast -> ACT
   copy to SBUF) hides under the V2 matmuls.
 - engine split per batch: ACT = exp(8) + h-gen(2) + G-evac(2) + V2-evac(2)
   + rden-evac; DVE = bn_stats + GN chain + recip + h-gen(2) + G-evac(2) +
   V2-evac(2) + PV-mul(4); Pool = residual adds (all-bf16 tensor_add; on
   the last batch they run on DVE to shorten the tail).
 - GroupNorm(b+1) DVE work is emitted between batch b's PV evacuations so
   the PSUM mega-pool rotation is never blocked behind 5us of bn_stats.
 - output is stored bf16 and cast to f32 on the host.
"""

import sys

for _p in ("/opt/trn_rl_repo", "/opt/trn_rl_repo/concourse"):
    if _p not in sys.path:
        sys.path.insert(0, _p)

import numpy as np
import ml_dtypes

import concourse.bass as bass
import concourse.mybir as mybir
import concourse.tile as tile
from concourse import bacc
from concourse.bass_utils import run_bass_kernel_spmd

F32 = mybir.dt.float32
BF16 = mybir.dt.bfloat16
FP8 = mybir.dt.float8e4
AOT = mybir.AluOpType
AFT = mybir.ActivationFunctionType
PM = mybir.MatmulPerfMode

P = 128          # partitions
C = 512          # channels
N = 1024         # tokens (H*W)
GROUPS = 32
EPS = 1e-5
NB = 4           # batch elements per core
CC = C // P      # 4 channel chunks
MC = N // P      # 8 token chunks
FD = 512         # PSUM bank free size (f32)
NHALF = N // FD  # 2
GSZ = C // GROUPS            # 16 channels per group
GPC = P // GSZ               # 8 groups per channel chunk
BSC = 8.0        # fp8-range scale on B (undone in exp scale)
W2SC = 8.0       # fp8-range scale on W2 (undone in rden constant)
ESHIFT = -1.0    # constant score shift before exp (cancels in softmax)


def build(reps: int = 1, nb: int = NB, debug: bool = False, with_z: bool = False,
          with_bias: bool = False, g_act: int = 4, v2_act: int = 4, h_act: int = 2,
          bn_head: bool = True, pv_pool: bool = True, unroll: int = 4):
    """Build the per-core Bass program. Identical on all 8 cores (SPMD over batch)."""
    nc = bacc.Bacc(None, target_bir_lowering=False)
    dbg = {}
    if debug:
        dbg["h"] = nc.dram_tensor("dbg_h", [P, CC, N], FP8, kind="ExternalOutput")
        dbg["g"] = nc.dram_tensor("dbg_g", [P, CC, N], FP8, kind="ExternalOutput")
        dbg["v"] = nc.dram_tensor("dbg_v", [P, MC, C], FP8, kind="ExternalOutput")
        dbg["e"] = nc.dram_tensor("dbg_e", [P, MC, N], FP8, kind="ExternalOutput")
        dbg["dn"] = nc.dram_tensor("dbg_dn", [1, N], F32, kind="ExternalOutput")

    x_d = nc.dram_tensor("xbf", [nb, C, N], BF16, kind="ExternalInput")
    b8_d = nc.dram_tensor("b8", [P, CC, C], FP8, kind="ExternalInput")
    w2t8_d = nc.dram_tensor("w2t8", [P, CC, C], FP8, kind="ExternalInput")
    # small per-channel params, host-packed: [P, 4*CC+GPC] f32 =
    # (gnsc | gnbi | btot | u_fold | a1)
    pf_d = nc.dram_tensor("pf32", [P, 4 * CC + GPC], F32, kind="ExternalInput")
    out_d = nc.dram_tensor("out", [nb, C, N], BF16, kind="ExternalOutput")

    a1 = np.zeros((P, GPC), np.float32)
    for p in range(P):
        a1[p, p // GSZ] = 1.0
    a2_d = nc.inline_tensor(np.ascontiguousarray(a1.T), name="a2")

    with tile.TileContext(nc) as tc:
        with (
            tc.tile_pool(name="wpool", bufs=1) as wpool,
            tc.tile_pool(name="xp", bufs=3) as xp,
            tc.tile_pool(name="hp", bufs=3) as hp,
            tc.tile_pool(name="qk", bufs=3) as qk,
            tc.tile_pool(name="vt", bufs=3) as vt,
            tc.tile_pool(name="ep", bufs=2) as ep,
            tc.tile_pool(name="rd", bufs=2) as rd,
            tc.tile_pool(name="fin", bufs=3) as fin,
            tc.tile_pool(name="gn", bufs=2) as gn,
            tc.tile_pool(name="ps", bufs=3, space="PSUM") as ps,
            tc.tile_pool(name="psd", bufs=1, space="PSUM") as psd,
        ):
            # ---- one-time per-core setup (small DMAs only) ----
            pf = wpool.tile([P, 4 * CC + GPC], F32, tag="pf")
            nc.sync.dma_start(out=pf[:], in_=pf_d[:])
            a2_sb = wpool.tile([GPC, P], F32, tag="a2")
            nc.sync.dma_start(out=a2_sb[:], in_=a2_d[:])
            gnsc, gnbi, btot, u8f = (pf[:, 4 * i:4 * i + 4] for i in range(4))
            a1_sb = pf[:, 4 * CC:4 * CC + GPC]
            eps_sb = wpool.tile([P, 1], F32, tag="eps")
            nc.vector.memset(eps_sb[:], EPS)
            esh_sb = wpool.tile([P, 1], F32, tag="esh")
            nc.vector.memset(esh_sb[:], ESHIFT)
            ones8 = wpool.tile([P, 2, 32], FP8, tag="ones8")
            nc.vector.memset(ones8[:], 1.0)
            onesf = wpool.tile([1, P], BF16, tag="onesf")
            nc.vector.memset(onesf[:], 1.0 / W2SC)
            onef = wpool.tile([1, 1], BF16, tag="onef")
            nc.vector.memset(onef[:], 1.0)
            b8 = wpool.tile([P, CC, C], FP8, tag="bmat")
            nc.sync.dma_start(out=b8[:], in_=b8_d[:])
            x_first = None
            if reps == 1:
                x_first = xp.tile([P, CC, N], BF16, tag="x")
                for cc in range(CC):
                    nc.sync.dma_start(out=x_first[:, cc, :],
                                      in_=x_d[0, cc * P:(cc + 1) * P, :])
            w2t8 = wpool.tile([P, CC, C], FP8, tag="w2t")
            nc.sync.dma_start(out=w2t8[:], in_=w2t8_d[:])
            u8 = None
            if with_z:
                u8 = wpool.tile([P, CC], FP8, tag="u8")
                nc.vector.tensor_copy(out=u8[:], in_=u8f[:])

            def load_x(b):
                if b == 0 and x_first is not None:
                    return x_first
                x_t = xp.tile([P, CC, N], BF16, tag="x")
                for cc in range(CC):
                    nc.sync.dma_start(out=x_t[:, cc, :],
                                      in_=x_d[b, cc * P:(cc + 1) * P, :])
                return x_t

            gn_state = {}

            def gn_stats(b, x_t, cc_list):
                # per-channel bn_stats for the given chunks (split so it can
                # interleave with the previous batch's PV evacuation on DVE)
                if b not in gn_state:
                    stats = gn.tile([P, CC, 2, 6], F32, tag="stats")
                    mv = gn.tile([P, CC, 2], F32, tag="mv")
                    gn_state[b] = (stats, mv)
                stats, mv = gn_state[b]
                for cc in cc_list:
                    for s in range(2):
                        nc.vector.bn_stats(out=stats[:, cc, s, :],
                                           in_=x_t[:, cc, s * 512:(s + 1) * 512])
                    nc.vector.bn_aggr(out=mv[:, cc, :], in_=stats[:, cc, :, :])
                return mv

            def gn_mid(b, mv):
                # E[x^2] fold and the cross-partition group matmuls; emitted
                # at a point where mv is already complete so the in-order PE
                # never stalls on the DVE round trip
                m2 = gn.tile([P, CC], F32, tag="m2")
                nc.vector.tensor_mul(out=m2[:], in0=mv[:, :, 0], in1=mv[:, :, 0])
                nc.vector.tensor_add(out=mv[:, :, 1], in0=mv[:, :, 1], in1=m2[:])
                gsp = ps.tile([P, N], F32, tag="mm")
                nc.tensor.matmul(gsp[:GPC, 0:2 * CC], lhsT=a1_sb[:],
                                 rhs=mv.rearrange("p a b -> p (a b)"),
                                 start=True, stop=True)
                gs_sb = gn.tile([GPC, 2 * CC], F32, tag="gs")
                nc.vector.tensor_copy(out=gs_sb[:], in_=gsp[:GPC, 0:2 * CC])
                bcp = ps.tile([P, N], F32, tag="mm")
                nc.tensor.matmul(bcp[:, 0:2 * CC], lhsT=a2_sb[:], rhs=gs_sb[:],
                                 start=True, stop=True)
                bc = gn.tile([P, CC, 2], F32, tag="bc")
                nc.vector.tensor_scalar(out=bc.rearrange("p a b -> p (a b)"),
                                        in0=bcp[:, 0:2 * CC], scalar1=1.0 / GSZ,
                                        scalar2=0.0, op0=AOT.mult, op1=AOT.add)
                return bc

            def gn_tail(b, x_t, bc):
                # var -> rstd (2 Newton iterations) -> gna/gnb -> h8 (fp8),
                # h-gen split 2 chunks DVE / 2 chunks ACT
                var = gn.tile([P, CC], F32, tag="var")
                nc.vector.tensor_mul(out=var[:], in0=bc[:, :, 0], in1=bc[:, :, 0])
                nc.vector.tensor_sub(out=var[:], in0=bc[:, :, 1], in1=var[:])
                nc.vector.tensor_scalar_add(var[:], var[:], eps_sb[:])
                rstd = gn.tile([P, CC], F32, tag="rstd")
                nc.vector.reciprocal(out=rstd[:], in_=var[:])
                t0 = gn.tile([P, CC], F32, tag="t0")
                for _ in range(2):
                    nc.vector.tensor_mul(out=t0[:], in0=rstd[:], in1=rstd[:])
                    nc.vector.tensor_mul(out=t0[:], in0=var[:], in1=t0[:])
                    nc.vector.tensor_scalar(out=t0[:], in0=t0[:], scalar1=-0.5,
                                            scalar2=1.5, op0=AOT.mult, op1=AOT.add)
                    nc.vector.tensor_mul(out=rstd[:], in0=rstd[:], in1=t0[:])
                gna = gn.tile([P, CC], F32, tag="gna")
                nc.vector.tensor_mul(out=gna[:], in0=rstd[:], in1=gnsc[:])
                gnb = gn.tile([P, CC], F32, tag="gnb")
                nc.vector.tensor_mul(out=gnb[:], in0=bc[:, :, 0], in1=gna[:])
                nc.vector.tensor_sub(out=gnb[:], in0=gnbi[:], in1=gnb[:])
                h8 = hp.tile([P, CC, N], FP8, tag="h")
                for cc in range(CC):
                    if cc >= CC - h_act:
                        nc.scalar.activation(
                            out=h8[:, cc, :], in_=x_t[:, cc, :], func=AFT.Identity,
                            scale=gna[:, cc:cc + 1], bias=gnb[:, cc:cc + 1])
                    else:
                        nc.vector.tensor_scalar(
                            out=h8[:, cc, :], in0=x_t[:, cc, :],
                            scalar1=gna[:, cc:cc + 1], scalar2=gnb[:, cc:cc + 1],
                            op0=AOT.mult, op1=AOT.add)
                if debug and b == 0:
                    nc.sync.dma_start(out=dbg["h"][:], in_=h8[:])
                return h8

            def groupnorm(b, x_t):
                mv = gn_stats(b, x_t, range(CC))
                return gn_tail(b, x_t, gn_mid(b, mv))

            def g_stage(b, h8):
                # G = B^T h8 (fp8 DoubleRow, icc pairs); one [P, N] mega tile
                # per jc, halves per nh; evac split ACT / DVE
                g8 = qk.tile([P, CC, N], FP8, tag="g")
                for jc in range(CC):
                    mm = ps.tile([P, N], F32, tag="mm")
                    for i in range(CC // 2):
                        for nh in range(NHALF):
                            nc.tensor.matmul(
                                mm[:, nh * FD:(nh + 1) * FD],
                                lhsT=b8[:, 2 * i:2 * i + 2, jc * P:(jc + 1) * P],
                                rhs=h8[:, 2 * i:2 * i + 2, nh * FD:(nh + 1) * FD],
                                start=(i == 0), stop=(i == CC // 2 - 1),
                                perf_mode=PM.DoubleRow, skip_group_check=True,
                            )
                    if jc < g_act:
                        nc.scalar.copy(out=g8[:, jc, :], in_=mm[:])
                    else:
                        nc.vector.tensor_copy(out=g8[:, jc, :], in_=mm[:])
                if debug and b == 0:
                    nc.sync.dma_start(out=dbg["g"][:], in_=g8[:])
                return g8

            def scores_stage(b, h8, g8):
                # S^T per mc in a [P, N] mega tile; exp -> e8 fp8 on ACT;
                # denominator column sums on PE (ones-fp8 DoubleRow)
                e8 = ep.tile([P, MC, N], FP8, tag="e")
                dn = psd.tile([32, N], F32, tag="dn")
                zb = None
                if with_z:
                    zrow = gn.tile([1, N], BF16, tag="zrow")
                    for mh in range(NHALF):
                        zp = ps.tile([P, N], F32, tag="mm")
                        for jcc in range(CC):
                            nc.tensor.matmul(
                                zp[:1, 0:FD],
                                lhsT=u8[:, jcc:jcc + 1],
                                rhs=h8[:, jcc, mh * FD:(mh + 1) * FD],
                                start=(jcc == 0), stop=(jcc == CC - 1),
                            )
                        nc.scalar.activation(out=zrow[:, mh * FD:(mh + 1) * FD],
                                             in_=zp[:1, 0:FD], func=AFT.Copy,
                                             scale=float(C) ** -0.5)
                    zbp = ps.tile([P, N], F32, tag="mm")
                    for mcq in range(MC):
                        nc.tensor.matmul(
                            zbp[:, mcq:mcq + 1],
                            lhsT=zrow[:1, mcq * P:(mcq + 1) * P],
                            rhs=onef[:1, :],
                            start=(mcq == 0), stop=(mcq == MC - 1),
                            skip_group_check=True,
                        )
                    zb = gn.tile([P, MC], F32, tag="zb")
                    nc.scalar.activation(out=zb[:], in_=zbp[:, 0:MC],
                                         func=AFT.Copy, bias=ESHIFT)
                escale = float(C) ** -0.5 / BSC
                for mc in range(MC):
                    mm = ps.tile([P, N], F32, tag="mm")
                    for i in range(CC // 2):
                        for nh in range(NHALF):
                            nc.tensor.matmul(
                                mm[:, nh * FD:(nh + 1) * FD],
                                lhsT=h8[:, 2 * i:2 * i + 2, mc * P:(mc + 1) * P],
                                rhs=g8[:, 2 * i:2 * i + 2, nh * FD:(nh + 1) * FD],
                                start=(i == 0), stop=(i == CC // 2 - 1),
                                perf_mode=PM.DoubleRow, skip_group_check=True,
                            )
                    nc.scalar.activation(
                        out=e8[:, mc, :], in_=mm[:],
                        func=AFT.Exp, scale=escale,
                        bias=(zb[:, mc:mc + 1] if with_z else esh_sb[:]),
                    )
                    if mc % 2 == 1:
                        j = mc // 2
                        for nh in range(NHALF):
                            nc.tensor.matmul(
                                dn[:, nh * FD:(nh + 1) * FD],
                                lhsT=ones8[:],
                                rhs=e8[:, 2 * j:2 * j + 2, nh * FD:(nh + 1) * FD],
                                start=(j == 0), stop=(j == MC // 2 - 1),
                                perf_mode=PM.DoubleRow,
                            )
                if debug and b == 0:
                    nc.sync.dma_start(out=dbg["e"][:], in_=e8[:])
                return e8, dn

            def recip_stage(b, dn):
                # 1/denom straight from PSUM on DVE, bf16 out for the
                # broadcast matmul
                if debug and b == 0:
                    dnc = rd.tile([1, N], F32, tag="dnc")
                    nc.vector.tensor_copy(out=dnc[:], in_=dn[:1, :])
                    nc.sync.dma_start(out=dbg["dn"][:], in_=dnc[:])
                rrb = rd.tile([1, N], BF16, tag="rrb")
                with nc.allow_low_precision(reason="rden in bf16: 0.4% on a 2e-2 budget"):
                    nc.vector.reciprocal(out=rrb[:], in_=dn[:1, :])
                return rrb

            def v2_stage(b, h8):
                # V2^T = h8^T W2^T (fp8 DoubleRow, cc pairs); [P, N] mega tile
                # per mc pair; evac split ACT / DVE
                vT8 = vt.tile([P, MC, C], FP8, tag="vT")
                for j in range(MC // 2):
                    mm = ps.tile([P, N], F32, tag="mm")
                    for half in range(2):
                        mc = 2 * j + half
                        for i in range(CC // 2):
                            nc.tensor.matmul(
                                mm[:, half * FD:(half + 1) * FD],
                                lhsT=h8[:, 2 * i:2 * i + 2, mc * P:(mc + 1) * P],
                                rhs=w2t8[:, 2 * i:2 * i + 2, :],
                                start=(i == 0), stop=(i == CC // 2 - 1),
                                perf_mode=PM.DoubleRow,
                            )
                    if j < v2_act:
                        nc.scalar.copy(out=vT8[:, 2 * j:2 * j + 2, :], in_=mm[:])
                    else:
                        nc.vector.tensor_copy(out=vT8[:, 2 * j:2 * j + 2, :], in_=mm[:])
                if debug and b == 0:
                    nc.sync.dma_start(out=dbg["v"][:], in_=vT8[:])
                return vT8

            def bcast_stage(b, rrb):
                # broadcast (1/W2SC)/denom across partitions: bf16 ones-matmul
                # into a mega psum slot, then one ACT copy -> bf16 SBUF
                bcp = ps.tile([P, N], F32, tag="mm")
                for nh in range(NHALF):
                    nc.tensor.matmul(
                        bcp[:, nh * FD:(nh + 1) * FD], lhsT=onesf[:],
                        rhs=rrb[:, nh * FD:(nh + 1) * FD],
                        start=True, stop=True, skip_group_check=True,
                    )
                rdb = rd.tile([P, N], BF16, tag="rdb")
                nc.vector.tensor_copy(out=rdb[:], in_=bcp[:])
                return rdb

            def pv_stage(b, x_t, vT8, e8, rdb, mid=None):
                # out = V2 attn^T (fp8 DoubleRow, mc pairs) in a [P, N] mega
                # tile per oc; evacuated raw by DMA (no engine time), then
                # Pool does psum*rden -> bf16 and the +x residual; the last
                # batch uses DVE instead to shorten the tail; store bf16
                f_t = fin.tile([P, CC, N], BF16, tag="f")
                last = (b == nb - 1)
                for oc in range(CC):
                    mm = ps.tile([P, N], F32, tag="mm")
                    for j in range(MC // 2):
                        for nh in range(NHALF):
                            nc.tensor.matmul(
                                mm[:, nh * FD:(nh + 1) * FD],
                                lhsT=vT8[:, 2 * j:2 * j + 2, oc * P:(oc + 1) * P],
                                rhs=e8[:, 2 * j:2 * j + 2, nh * FD:(nh + 1) * FD],
                                start=(j == 0), stop=(j == MC // 2 - 1),
                                perf_mode=PM.DoubleRow, skip_group_check=True,
                            )
                    nc.vector.tensor_mul(out=f_t[:, oc, :], in0=mm[:], in1=rdb[:])
                    if with_bias:
                        nc.vector.scalar_tensor_tensor(
                            out=f_t[:, oc, :], in0=f_t[:, oc, :],
                            scalar=btot[:, oc:oc + 1],
                            in1=x_t[:, oc, :], op0=AOT.add, op1=AOT.add)
                    elif last or not pv_pool:
                        nc.vector.tensor_add(out=f_t[:, oc, :],
                                             in0=f_t[:, oc, :], in1=x_t[:, oc, :])
                    else:
                        nc.gpsimd.tensor_add(out=f_t[:, oc, :],
                                             in0=f_t[:, oc, :], in1=x_t[:, oc, :])
                    nc.sync.dma_start(out=out_d[b, oc * P:(oc + 1) * P, :],
                                      in_=f_t[:, oc, :])
                    if mid is not None and oc == 1:
                        mid()

            def body_all(_i=None):
                gn_state.clear()
                x_t = load_x(0)
                h8 = groupnorm(0, x_t)
                state = (x_t, h8)
                for b in range(nb):
                    x_t, h8 = state
                    x_next = load_x(b + 1) if b + 1 < nb else None
                    g8 = g_stage(b, h8)
                    e8, dn = scores_stage(b, h8, g8)
                    # recip first in the DVE queue so the rden chain never
                    # waits behind next-batch bn_stats
                    rrb = recip_stage(b, dn)
                    mv_n = gn_stats(b + 1, x_next, range(CC)) if x_next is not None else None
                    # group matmuls hit the PE only after mv is long ready
                    bc_n = gn_mid(b + 1, mv_n) if x_next is not None else None
                    vT8 = v2_stage(b, h8)
                    rdb = bcast_stage(b, rrb)
                    holder = {}
                    mid = None
                    if x_next is not None:
                        def mid(xn=x_next, bb=b + 1, bc=bc_n):
                            holder["h8"] = gn_tail(bb, xn, bc)
                    pv_stage(b, x_t, vT8, e8, rdb, mid)
                    if x_next is not None:
                        state = (x_next, holder["h8"])

            if reps == 1:
                body_all()
            elif reps < 0:
                for _ in range(-reps):
                    body_all()
            else:
                # multiple bodies per For_i iteration: amortizes the loop's
                # all-engine barrier and lets consecutive bodies overlap
                assert reps % unroll == 0
                with tc.For_i(0, reps // unroll, 1):
                    for _ in range(unroll):
                        body_all()

    nc.finalize()
    return nc


_NC_CACHE = {}


def _get_nc(reps: int = 1, with_z: bool = False, with_bias: bool = False):
    key = (reps, with_z, with_bias)
    if key not in _NC_CACHE:
        _NC_CACHE[key] = build(reps, with_z=with_z, with_bias=with_bias)
    return _NC_CACHE[key]


def _chunked(a):
    # [C, X] -> [P, CC, X] with c = cc*P + p
    return np.ascontiguousarray(np.transpose(a.reshape(CC, P, -1), (1, 0, 2)))


def _prep_in_maps(x, gn_scale, gn_bias, wq, bq, wk, bk, wv, bv, wproj, bproj, nb=NB):
    x = np.asarray(x, np.float32).reshape(-1, C, N)
    n_cores = x.shape[0] // nb

    def packb(v):
        return np.ascontiguousarray(np.asarray(v, np.float32).reshape(CC, P).T)

    wq = np.asarray(wq, np.float32)
    wk = np.asarray(wk, np.float32)
    wv = np.asarray(wv, np.float32)
    wproj = np.asarray(wproj, np.float32)
    B = wq.T @ wk
    W2T = (wproj @ wv).T
    b8 = _chunked(B * BSC).astype(ml_dtypes.float8_e4m3fn)
    w2t8 = _chunked(W2T * W2SC).astype(ml_dtypes.float8_e4m3fn)
    btot = np.asarray(bproj, np.float32) + wproj @ np.asarray(bv, np.float32)
    ufold = wk.T @ np.asarray(bq, np.float32)

    a1 = np.zeros((P, GPC), np.float32)
    for p in range(P):
        a1[p, p // GSZ] = 1.0
    pf32 = np.ascontiguousarray(np.concatenate(
        [packb(gn_scale), packb(gn_bias), packb(btot), packb(ufold), a1],
        axis=1))
    xbf = x.astype(ml_dtypes.bfloat16)
    common = {"b8": b8, "w2t8": w2t8, "pf32": pf32}
    in_maps = []
    for core in range(n_cores):
        m = dict(common)
        m["xbf"] = np.ascontiguousarray(xbf[core * nb:(core + 1) * nb])
        in_maps.append(m)
    return in_maps


def kernel(x, gn_scale, gn_bias, wq, bq, wk, bk, wv, bv, wproj, bproj):
    with_z = bool(np.any(np.asarray(bq, np.float32)))
    with_bias = bool(np.any(np.asarray(bproj, np.float32))) or bool(
        np.any(np.asarray(bv, np.float32)))
    nc = _get_nc(1, with_z=with_z, with_bias=with_bias)
    in_maps = _prep_in_maps(x, gn_scale, gn_bias, wq, bq, wk, bk, wv, bv, wproj, bproj)
    res = run_bass_kernel_spmd(nc, in_maps, core_ids=list(range(8)))
    out = np.concatenate([np.asarray(res.results[i]["out"]) for i in range(8)], axis=0)
    return out.reshape(32, C, 32, 32).astype(np.float32)


# revision 7
# speedup vs baseline: 1.1108x; 1.1108x over previous
"""Trainium2 Bass kernel for nn_AttentionBlock — fp8 DoubleRow version (v3).

Sharding: data-parallel over batch B=32 across 8 NeuronCores (4 per core).

vs the bf16 baseline (245us): all four per-batch matmul stages (G = B^T h,
V2^T = h^T W2^T, S^T = h^T G, out = V2 attn^T) run in fp8e4m3 with the
DoubleRow perf mode — each instruction contracts 256 (two 128-chunks packed
into the stationary) at 0.5 cycles/row, ~4x the bf16 MAC rate (measured
~240ns vs ~920ns per K=512 x [128,512] group). The 2e-2 rel gate has ~50x
headroom over bf16 (3.6e-4); measured fp8 end-to-end error is ~5e-3.

Measured ladder (slope method, same contract as the baseline's 245us):
bf16 baseline 237 -> fp8 naive 183 -> host-folded B/W2 + mega-tiles +
reorder 152 -> For_i 4-body unrolling (the tile For_i has an all-engine
barrier per iteration; unrolling amortizes it) + pipeline-depth fixes +
ACT->DVE offloads (rden-broadcast evac, GroupNorm bc scale, one h-gen
chunk) -> 149.7-170us depending on device power state (the per-iteration
slope itself drifts ~13% between states). Rel err 5.1e-3 (gate 2e-2).
ACT (exp + the PSUM mega-evacuations) is the hardware wall; same-process
A/B showed the evacuations must stay on ACT (moving one V2-evac to DVE
costs 13%, one G-evac 3.5% — DVE queue-order collisions), while exactly
one of the four h-gen chunks on ACT is optimal (h_act=0 costs 7%,
h_act=2 costs 0.7%).

Structure (driven by TimelineSim: the kernel is stall- and
evacuation-bound, not matmul-bound, after fp8; GPSIMD cannot touch PSUM
and DMA cannot read PSUM, so all PSUM evacuation goes through ACT/DVE):
 - B = wq^T wk (xBSC) and W2^T = (wproj wv)^T (xW2SC) are computed on the
   HOST and shipped as fp8 — removes all setup matmuls, their PSUM
   evacuations, and 1.5MB of weight DMA from the kernel head.
 - x arrives as bf16 (halves input DMA; residual in bf16 is within budget).
 - every PSUM tile is a [P, 1024] 2-bank mega-tile (one evacuation
   instruction each): pool layout = 3x mega (6 banks) + denominator (2).
 - per-batch PE order: G -> S/exp (+ denominator ones-fp8-DoubleRow matmuls
   interleaved) -> V2 -> rden broadcast -> PV. The denominator chain
   (recip on DVE directly from PSUM -> bf16 ones-matmul broadcast -> ACT
   copy to SBUF) hides under the V2 matmuls.
 - engine split per batch: ACT = exp(8) + all G-evacs(4) + all V2-evacs(4)
   + h-gen(1); DVE = bn_stats + GN chain + recip + rden-broadcast evac +
   h-gen(3) + PV-mul(4); Pool = residual adds (all-bf16 tensor_add; on the
   last batch they run on DVE to shorten the tail).
 - GroupNorm(b+1) is pipelined across batch b: bn_stats after the
   reciprocal in the DVE queue, the group matmuls between S and V2 on PE
   (inputs long ready, so the in-order PE never stalls on the DVE round
   trip), and the tail/h-gen emitted mid-PV via a callback.
 - output is stored bf16 and cast to f32 on the host.
"""

import sys

for _p in ("/opt/trn_rl_repo", "/opt/trn_rl_repo/concourse"):
    if _p not in sys.path:
        sys.path.insert(0, _p)

import numpy as np
import ml_dtypes

import concourse.bass as bass
import concourse.mybir as mybir
import concourse.tile as tile
from concourse import bacc
from concourse.bass_utils import run_bass_kernel_spmd

F32 = mybir.dt.float32
BF16 = mybir.dt.bfloat16
FP8 = mybir.dt.float8e4
AOT = mybir.AluOpType
AFT = mybir.ActivationFunctionType
PM = mybir.MatmulPerfMode

P = 128          # partitions
C = 512          # channels
N = 1024         # tokens (H*W)
GROUPS = 32
EPS = 1e-5
NB = 4           # batch elements per core
CC = C // P      # 4 channel chunks
MC = N // P      # 8 token chunks
FD = 512         # PSUM bank free size (f32)
NHALF = N // FD  # 2
GSZ = C // GROUPS            # 16 channels per group
GPC = P // GSZ               # 8 groups per channel chunk
BSC = 8.0        # fp8-range scale on B (undone in exp scale)
W2SC = 8.0       # fp8-range scale on W2 (undone in rden constant)
ESHIFT = -1.0    # constant score shift before exp (cancels in softmax)


def build(reps: int = 1, nb: int = NB, debug: bool = False, with_z: bool = False,
          with_bias: bool = False, g_act: int = 4, v2_act: int = 4, h_act: int = 1,
          bn_head: bool = True, pv_pool: bool = True, unroll: int = 4,
          noshift: bool = True):
    """Build the per-core Bass program. Identical on all 8 cores (SPMD over batch)."""
    nc = bacc.Bacc(None, target_bir_lowering=False)
    dbg = {}
    if debug:
        dbg["h"] = nc.dram_tensor("dbg_h", [P, CC, N], FP8, kind="ExternalOutput")
        dbg["g"] = nc.dram_tensor("dbg_g", [P, CC, N], FP8, kind="ExternalOutput")
        dbg["v"] = nc.dram_tensor("dbg_v", [P, MC, C], FP8, kind="ExternalOutput")
        dbg["e"] = nc.dram_tensor("dbg_e", [P, MC, N], FP8, kind="ExternalOutput")
        dbg["dn"] = nc.dram_tensor("dbg_dn", [1, N], F32, kind="ExternalOutput")

    x_d = nc.dram_tensor("xbf", [nb, C, N], BF16, kind="ExternalInput")
    b8_d = nc.dram_tensor("b8", [P, CC, C], FP8, kind="ExternalInput")
    w2t8_d = nc.dram_tensor("w2t8", [P, CC, C], FP8, kind="ExternalInput")
    # small per-channel params, host-packed: [P, 4*CC+GPC] f32 =
    # (gnsc | gnbi | btot | u_fold | a1)
    pf_d = nc.dram_tensor("pf32", [P, 4 * CC + GPC], F32, kind="ExternalInput")
    out_d = nc.dram_tensor("out", [nb, C, N], BF16, kind="ExternalOutput")

    a1 = np.zeros((P, GPC), np.float32)
    for p in range(P):
        a1[p, p // GSZ] = 1.0
    a2_d = nc.inline_tensor(np.ascontiguousarray(a1.T), name="a2")

    with tile.TileContext(nc) as tc:
        with (
            tc.tile_pool(name="wpool", bufs=1) as wpool,
            tc.tile_pool(name="xp", bufs=3) as xp,
            tc.tile_pool(name="hp", bufs=3) as hp,
            tc.tile_pool(name="qk", bufs=3) as qk,
            tc.tile_pool(name="vt", bufs=3) as vt,
            tc.tile_pool(name="ep", bufs=2) as ep,
            tc.tile_pool(name="rd", bufs=2) as rd,
            tc.tile_pool(name="fin", bufs=3) as fin,
            tc.tile_pool(name="gn", bufs=2) as gn,
            tc.tile_pool(name="ps", bufs=3, space="PSUM") as ps,
            tc.tile_pool(name="psd", bufs=1, space="PSUM") as psd,
        ):
            # ---- one-time per-core setup (small DMAs only) ----
            pf = wpool.tile([P, 4 * CC + GPC], F32, tag="pf")
            nc.sync.dma_start(out=pf[:], in_=pf_d[:])
            a2_sb = wpool.tile([GPC, P], F32, tag="a2")
            nc.sync.dma_start(out=a2_sb[:], in_=a2_d[:])
            gnsc, gnbi, btot, u8f = (pf[:, 4 * i:4 * i + 4] for i in range(4))
            a1_sb = pf[:, 4 * CC:4 * CC + GPC]
            eps_sb = wpool.tile([P, 1], F32, tag="eps")
            nc.vector.memset(eps_sb[:], EPS)
            esh_sb = wpool.tile([P, 1], F32, tag="esh")
            nc.vector.memset(esh_sb[:], ESHIFT)
            ones8 = wpool.tile([P, 2, 32], FP8, tag="ones8")
            nc.vector.memset(ones8[:], 1.0)
            onesf = wpool.tile([1, P], BF16, tag="onesf")
            nc.vector.memset(onesf[:], 1.0 / W2SC)
            onef = wpool.tile([1, 1], BF16, tag="onef")
            nc.vector.memset(onef[:], 1.0)
            b8 = wpool.tile([P, CC, C], FP8, tag="bmat")
            nc.sync.dma_start(out=b8[:], in_=b8_d[:])
            x_first = None
            if reps == 1:
                x_first = xp.tile([P, CC, N], BF16, tag="x")
                for cc in range(CC):
                    nc.sync.dma_start(out=x_first[:, cc, :],
                                      in_=x_d[0, cc * P:(cc + 1) * P, :])
            w2t8 = wpool.tile([P, CC, C], FP8, tag="w2t")
            nc.sync.dma_start(out=w2t8[:], in_=w2t8_d[:])
            u8 = None
            if with_z:
                u8 = wpool.tile([P, CC], FP8, tag="u8")
                nc.vector.tensor_copy(out=u8[:], in_=u8f[:])

            def load_x(b):
                if b == 0 and x_first is not None:
                    return x_first
                x_t = xp.tile([P, CC, N], BF16, tag="x")
                for cc in range(CC):
                    nc.sync.dma_start(out=x_t[:, cc, :],
                                      in_=x_d[b, cc * P:(cc + 1) * P, :])
                return x_t

            gn_state = {}

            def gn_stats(b, x_t, cc_list):
                # per-channel bn_stats for the given chunks (split so it can
                # interleave with the previous batch's PV evacuation on DVE)
                if b not in gn_state:
                    stats = gn.tile([P, CC, 2, 6], F32, tag="stats")
                    mv = gn.tile([P, CC, 2], F32, tag="mv")
                    gn_state[b] = (stats, mv)
                stats, mv = gn_state[b]
                for cc in cc_list:
                    for s in range(2):
                        nc.vector.bn_stats(out=stats[:, cc, s, :],
                                           in_=x_t[:, cc, s * 512:(s + 1) * 512])
                    nc.vector.bn_aggr(out=mv[:, cc, :], in_=stats[:, cc, :, :])
                return mv

            def gn_mid(b, mv):
                # E[x^2] fold and the cross-partition group matmuls; emitted
                # at a point where mv is already complete so the in-order PE
                # never stalls on the DVE round trip
                m2 = gn.tile([P, CC], F32, tag="m2")
                nc.vector.tensor_mul(out=m2[:], in0=mv[:, :, 0], in1=mv[:, :, 0])
                nc.vector.tensor_add(out=mv[:, :, 1], in0=mv[:, :, 1], in1=m2[:])
                gsp = ps.tile([P, N], F32, tag="mm")
                nc.tensor.matmul(gsp[:GPC, 0:2 * CC], lhsT=a1_sb[:],
                                 rhs=mv.rearrange("p a b -> p (a b)"),
                                 start=True, stop=True)
                gs_sb = gn.tile([GPC, 2 * CC], F32, tag="gs")
                nc.vector.tensor_copy(out=gs_sb[:], in_=gsp[:GPC, 0:2 * CC])
                bcp = ps.tile([P, N], F32, tag="mm")
                nc.tensor.matmul(bcp[:, 0:2 * CC], lhsT=a2_sb[:], rhs=gs_sb[:],
                                 start=True, stop=True)
                bc = gn.tile([P, CC, 2], F32, tag="bc")
                nc.vector.tensor_scalar(out=bc.rearrange("p a b -> p (a b)"),
                                        in0=bcp[:, 0:2 * CC], scalar1=1.0 / GSZ,
                                        scalar2=0.0, op0=AOT.mult, op1=AOT.add)
                return bc

            def gn_tail(b, x_t, bc):
                # var -> rstd (2 Newton iterations) -> gna/gnb -> h8 (fp8),
                # h-gen split 2 chunks DVE / 2 chunks ACT
                var = gn.tile([P, CC], F32, tag="var")
                nc.vector.tensor_mul(out=var[:], in0=bc[:, :, 0], in1=bc[:, :, 0])
                nc.vector.tensor_sub(out=var[:], in0=bc[:, :, 1], in1=var[:])
                nc.vector.tensor_scalar_add(var[:], var[:], EPS)
                rstd = gn.tile([P, CC], F32, tag="rstd")
                nc.vector.reciprocal(out=rstd[:], in_=var[:])
                t0 = gn.tile([P, CC], F32, tag="t0")
                for _ in range(2):
                    nc.vector.tensor_mul(out=t0[:], in0=rstd[:], in1=rstd[:])
                    nc.vector.tensor_mul(out=t0[:], in0=var[:], in1=t0[:])
                    nc.vector.tensor_scalar(out=t0[:], in0=t0[:], scalar1=-0.5,
                                            scalar2=1.5, op0=AOT.mult, op1=AOT.add)
                    nc.vector.tensor_mul(out=rstd[:], in0=rstd[:], in1=t0[:])
                gna = gn.tile([P, CC], F32, tag="gna")
                nc.vector.tensor_mul(out=gna[:], in0=rstd[:], in1=gnsc[:])
                gnb = gn.tile([P, CC], F32, tag="gnb")
                nc.vector.tensor_mul(out=gnb[:], in0=bc[:, :, 0], in1=gna[:])
                nc.vector.tensor_sub(out=gnb[:], in0=gnbi[:], in1=gnb[:])
                h8 = hp.tile([P, CC, N], FP8, tag="h")
                for cc in range(CC):
                    if cc >= CC - h_act:
                        nc.scalar.activation(
                            out=h8[:, cc, :], in_=x_t[:, cc, :], func=AFT.Identity,
                            scale=gna[:, cc:cc + 1], bias=gnb[:, cc:cc + 1])
                    else:
                        nc.vector.tensor_scalar(
                            out=h8[:, cc, :], in0=x_t[:, cc, :],
                            scalar1=gna[:, cc:cc + 1], scalar2=gnb[:, cc:cc + 1],
                            op0=AOT.mult, op1=AOT.add)
                if debug and b == 0:
                    nc.sync.dma_start(out=dbg["h"][:], in_=h8[:])
                return h8

            def groupnorm(b, x_t):
                mv = gn_stats(b, x_t, range(CC))
                return gn_tail(b, x_t, gn_mid(b, mv))

            def g_stage(b, h8):
                # G = B^T h8 (fp8 DoubleRow, icc pairs); one [P, N] mega tile
                # per jc, halves per nh; evac split ACT / DVE
                g8 = qk.tile([P, CC, N], FP8, tag="g")
                for jc in range(CC):
                    mm = ps.tile([P, N], F32, tag="mm")
                    for i in range(CC // 2):
                        for nh in range(NHALF):
                            nc.tensor.matmul(
                                mm[:, nh * FD:(nh + 1) * FD],
                                lhsT=b8[:, 2 * i:2 * i + 2, jc * P:(jc + 1) * P],
                                rhs=h8[:, 2 * i:2 * i + 2, nh * FD:(nh + 1) * FD],
                                start=(i == 0), stop=(i == CC // 2 - 1),
                                perf_mode=PM.DoubleRow, skip_group_check=True,
                            )
                    if jc < g_act:
                        nc.scalar.copy(out=g8[:, jc, :], in_=mm[:])
                    else:
                        nc.vector.tensor_copy(out=g8[:, jc, :], in_=mm[:])
                if debug and b == 0:
                    nc.sync.dma_start(out=dbg["g"][:], in_=g8[:])
                return g8

            def scores_stage(b, h8, g8):
                # S^T per mc in a [P, N] mega tile; exp -> e8 fp8 on ACT;
                # denominator column sums on PE (ones-fp8 DoubleRow)
                e8 = ep.tile([P, MC, N], FP8, tag="e")
                dn = psd.tile([32, N], F32, tag="dn")
                zb = None
                if with_z:
                    zrow = gn.tile([1, N], BF16, tag="zrow")
                    for mh in range(NHALF):
                        zp = ps.tile([P, N], F32, tag="mm")
                        for jcc in range(CC):
                            nc.tensor.matmul(
                                zp[:1, 0:FD],
                                lhsT=u8[:, jcc:jcc + 1],
                                rhs=h8[:, jcc, mh * FD:(mh + 1) * FD],
                                start=(jcc == 0), stop=(jcc == CC - 1),
                            )
                        nc.scalar.activation(out=zrow[:, mh * FD:(mh + 1) * FD],
                                             in_=zp[:1, 0:FD], func=AFT.Copy,
                                             scale=float(C) ** -0.5)
                    zbp = ps.tile([P, N], F32, tag="mm")
                    for mcq in range(MC):
                        nc.tensor.matmul(
                            zbp[:, mcq:mcq + 1],
                            lhsT=zrow[:1, mcq * P:(mcq + 1) * P],
                            rhs=onef[:1, :],
                            start=(mcq == 0), stop=(mcq == MC - 1),
                            skip_group_check=True,
                        )
                    zb = gn.tile([P, MC], F32, tag="zb")
                    nc.scalar.activation(out=zb[:], in_=zbp[:, 0:MC],
                                         func=AFT.Copy, bias=ESHIFT)
                escale = float(C) ** -0.5 / BSC
                for mc in range(MC):
                    mm = ps.tile([P, N], F32, tag="mm")
                    for i in range(CC // 2):
                        for nh in range(NHALF):
                            nc.tensor.matmul(
                                mm[:, nh * FD:(nh + 1) * FD],
                                lhsT=h8[:, 2 * i:2 * i + 2, mc * P:(mc + 1) * P],
                                rhs=g8[:, 2 * i:2 * i + 2, nh * FD:(nh + 1) * FD],
                                start=(i == 0), stop=(i == CC // 2 - 1),
                                perf_mode=PM.DoubleRow, skip_group_check=True,
                            )
                    nc.scalar.activation(
                        out=e8[:, mc, :], in_=mm[:],
                        func=AFT.Exp, scale=escale,
                        bias=(zb[:, mc:mc + 1] if with_z
                              else (0.0 if noshift else esh_sb[:])),
                    )
                    if mc % 2 == 1:
                        j = mc // 2
                        for nh in range(NHALF):
                            nc.tensor.matmul(
                                dn[:, nh * FD:(nh + 1) * FD],
                                lhsT=ones8[:],
                                rhs=e8[:, 2 * j:2 * j + 2, nh * FD:(nh + 1) * FD],
                                start=(j == 0), stop=(j == MC // 2 - 1),
                                perf_mode=PM.DoubleRow,
                            )
                if debug and b == 0:
                    nc.sync.dma_start(out=dbg["e"][:], in_=e8[:])
                return e8, dn

            def recip_stage(b, dn):
                # 1/denom straight from PSUM on DVE, bf16 out for the
                # broadcast matmul
                if debug and b == 0:
                    dnc = rd.tile([1, N], F32, tag="dnc")
                    nc.vector.tensor_copy(out=dnc[:], in_=dn[:1, :])
                    nc.sync.dma_start(out=dbg["dn"][:], in_=dnc[:])
                rrb = rd.tile([1, N], BF16, tag="rrb")
                with nc.allow_low_precision(reason="rden in bf16: 0.4% on a 2e-2 budget"):
                    nc.vector.reciprocal(out=rrb[:], in_=dn[:1, :])
                return rrb

            def v2_stage(b, h8):
                # V2^T = h8^T W2^T (fp8 DoubleRow, cc pairs); [P, N] mega tile
                # per mc pair; evac split ACT / DVE
                vT8 = vt.tile([P, MC, C], FP8, tag="vT")
                for j in range(MC // 2):
                    mm = ps.tile([P, N], F32, tag="mm")
                    for half in range(2):
                        mc = 2 * j + half
                        for i in range(CC // 2):
                            nc.tensor.matmul(
                                mm[:, half * FD:(half + 1) * FD],
                                lhsT=h8[:, 2 * i:2 * i + 2, mc * P:(mc + 1) * P],
                                rhs=w2t8[:, 2 * i:2 * i + 2, :],
                                start=(i == 0), stop=(i == CC // 2 - 1),
                                perf_mode=PM.DoubleRow,
                            )
                    if j < v2_act:
                        nc.scalar.copy(out=vT8[:, 2 * j:2 * j + 2, :], in_=mm[:])
                    else:
                        nc.vector.tensor_copy(out=vT8[:, 2 * j:2 * j + 2, :], in_=mm[:])
                if debug and b == 0:
                    nc.sync.dma_start(out=dbg["v"][:], in_=vT8[:])
                return vT8

            def bcast_stage(b, rrb):
                # broadcast (1/W2SC)/denom across partitions: bf16 ones-matmul
                # into a mega psum slot, then one ACT copy -> bf16 SBUF
                bcp = ps.tile([P, N], F32, tag="mm")
                for nh in range(NHALF):
                    nc.tensor.matmul(
                        bcp[:, nh * FD:(nh + 1) * FD], lhsT=onesf[:],
                        rhs=rrb[:, nh * FD:(nh + 1) * FD],
                        start=True, stop=True, skip_group_check=True,
                    )
                rdb = rd.tile([P, N], BF16, tag="rdb")
                nc.vector.tensor_copy(out=rdb[:], in_=bcp[:])
                return rdb

            def pv_stage(b, x_t, vT8, e8, rdb, mid=None):
                # out = V2 attn^T (fp8 DoubleRow, mc pairs) in a [P, N] mega
                # tile per oc; evacuated raw by DMA (no engine time), then
                # Pool does psum*rden -> bf16 and the +x residual; the last
                # batch uses DVE instead to shorten the tail; store bf16
                f_t = fin.tile([P, CC, N], BF16, tag="f")
                last = (b == nb - 1)
                for oc in range(CC):
                    mm = ps.tile([P, N], F32, tag="mm")
                    for j in range(MC // 2):
                        for nh in range(NHALF):
                            nc.tensor.matmul(
                                mm[:, nh * FD:(nh + 1) * FD],
                                lhsT=vT8[:, 2 * j:2 * j + 2, oc * P:(oc + 1) * P],
                                rhs=e8[:, 2 * j:2 * j + 2, nh * FD:(nh + 1) * FD],
                                start=(j == 0), stop=(j == MC // 2 - 1),
                                perf_mode=PM.DoubleRow, skip_group_check=True,
                            )
                    nc.vector.tensor_mul(out=f_t[:, oc, :], in0=mm[:], in1=rdb[:])
                    if with_bias:
                        nc.vector.scalar_tensor_tensor(
                            out=f_t[:, oc, :], in0=f_t[:, oc, :],
                            scalar=btot[:, oc:oc + 1],
                            in1=x_t[:, oc, :], op0=AOT.add, op1=AOT.add)
                    elif last or not pv_pool:
                        nc.vector.tensor_add(out=f_t[:, oc, :],
                                             in0=f_t[:, oc, :], in1=x_t[:, oc, :])
                    else:
                        nc.gpsimd.tensor_add(out=f_t[:, oc, :],
                                             in0=f_t[:, oc, :], in1=x_t[:, oc, :])
                    nc.sync.dma_start(out=out_d[b, oc * P:(oc + 1) * P, :],
                                      in_=f_t[:, oc, :])
                    if mid is not None and oc == 1:
                        mid()

            def body_all(_i=None):
                gn_state.clear()
                x_t = load_x(0)
                h8 = groupnorm(0, x_t)
                state = (x_t, h8)
                for b in range(nb):
                    x_t, h8 = state
                    x_next = load_x(b + 1) if b + 1 < nb else None
                    g8 = g_stage(b, h8)
                    e8, dn = scores_stage(b, h8, g8)
                    # recip first in the DVE queue so the rden chain never
                    # waits behind next-batch bn_stats
                    rrb = recip_stage(b, dn)
                    mv_n = gn_stats(b + 1, x_next, range(CC)) if x_next is not None else None
                    # group matmuls hit the PE only after mv is long ready
                    bc_n = gn_mid(b + 1, mv_n) if x_next is not None else None
                    vT8 = v2_stage(b, h8)
                    rdb = bcast_stage(b, rrb)
                    holder = {}
                    mid = None
                    if x_next is not None:
                        def mid(xn=x_next, bb=b + 1, bc=bc_n):
                            holder["h8"] = gn_tail(bb, xn, bc)
                    pv_stage(b, x_t, vT8, e8, rdb, mid)
                    if x_next is not None:
                        state = (x_next, holder["h8"])

            if reps == 1:
                body_all()
            elif reps < 0:
                for _ in range(-reps):
                    body_all()
            else:
                # multiple bodies per For_i iteration: amortizes the loop's
                # all-engine barrier and lets consecutive bodies overlap
                assert reps % unroll == 0
                with tc.For_i(0, reps // unroll, 1):
                    for _ in range(unroll):
                        body_all()

    nc.finalize()
    return nc


_NC_CACHE = {}


def _get_nc(reps: int = 1, with_z: bool = False, with_bias: bool = False):
    key = (reps, with_z, with_bias)
    if key not in _NC_CACHE:
        _NC_CACHE[key] = build(reps, with_z=with_z, with_bias=with_bias)
    return _NC_CACHE[key]


def _chunked(a):
    # [C, X] -> [P, CC, X] with c = cc*P + p
    return np.ascontiguousarray(np.transpose(a.reshape(CC, P, -1), (1, 0, 2)))


def _prep_in_maps(x, gn_scale, gn_bias, wq, bq, wk, bk, wv, bv, wproj, bproj, nb=NB):
    x = np.asarray(x, np.float32).reshape(-1, C, N)
    n_cores = x.shape[0] // nb

    def packb(v):
        return np.ascontiguousarray(np.asarray(v, np.float32).reshape(CC, P).T)

    wq = np.asarray(wq, np.float32)
    wk = np.asarray(wk, np.float32)
    wv = np.asarray(wv, np.float32)
    wproj = np.asarray(wproj, np.float32)
    B = wq.T @ wk
    W2T = (wproj @ wv).T
    b8 = _chunked(B * BSC).astype(ml_dtypes.float8_e4m3fn)
    w2t8 = _chunked(W2T * W2SC).astype(ml_dtypes.float8_e4m3fn)
    btot = np.asarray(bproj, np.float32) + wproj @ np.asarray(bv, np.float32)
    ufold = wk.T @ np.asarray(bq, np.float32)

    a1 = np.zeros((P, GPC), np.float32)
    for p in range(P):
        a1[p, p // GSZ] = 1.0
    pf32 = np.ascontiguousarray(np.concatenate(
        [packb(gn_scale), packb(gn_bias), packb(btot), packb(ufold), a1],
        axis=1))
    xbf = x.astype(ml_dtypes.bfloat16)
    common = {"b8": b8, "w2t8": w2t8, "pf32": pf32}
    in_maps = []
    for core in range(n_cores):
        m = dict(common)
        m["xbf"] = np.ascontiguousarray(xbf[core * nb:(core + 1) * nb])
        in_maps.append(m)
    return in_maps


def kernel(x, gn_scale, gn_bias, wq, bq, wk, bk, wv, bv, wproj, bproj):
    with_z = bool(np.any(np.asarray(bq, np.float32)))
    with_bias = bool(np.any(np.asarray(bproj, np.float32))) or bool(
        np.any(np.asarray(bv, np.float32)))
    nc = _get_nc(1, with_z=with_z, with_bias=with_bias)
    in_maps = _prep_in_maps(x, gn_scale, gn_bias, wq, bq, wk, bk, wv, bv, wproj, bproj)
    res = run_bass_kernel_spmd(nc, in_maps, core_ids=list(range(8)))
    out = np.concatenate([np.asarray(res.results[i]["out"]) for i in range(8)], axis=0)
    return out.reshape(32, C, 32, 32).astype(np.float32)
